# revision 1
# baseline (speedup 1.0000x reference)
"""YOLO-style loss kernel for Trainium2, 8-core data parallel.

Per cell (B=1, C=21; conf target is exactly 0.0 or 1.0):
  sigma(v)  = 0.5*(1 + tanh(v/2))               (Tanh in same ACT table set as Exp)
  q         = sigma(p) - t ; e = q^2
  loss_xy   = sum(conf * e_x) ; loss_obj = sum(conf * e_c) ; e_tot = sum(e_c)
  loss_wh   = sum(conf * (p1 - ln(w_t/SCALE))^2)   (deferred Ln chunks)
  class: s = sum_c exp(l_c) ; t1 = sum_c exp(l_c)*(2+0.5c) = s + N
    d0 = t1 / (s*(ct+1)) ; diff = 10*d0 - 10 ; a = |diff|*cmask ; r = relu(a-1)
    huber*cmask = 0.5*(a^2 - r^2) ; cmask = min(ct,1)*conf
  el/prod reduced over 21 classes via bf16 binary add tree (DVE 2x mode).
  e_tot accumulated on ACT via Square+accum_out; other sums via DVE
  tensor_tensor_reduce with seed-chained accumulators.
  Work is software-pipelined over 6 stages so per-engine instruction
  streams are wait-free in steady state (each op's inputs were produced
  in an earlier iteration or early in the same iteration).
Per-core output: [128, 8] partial sums (xy, wh, obj, e_total, cls, 0..).
"""

import numpy as np

S = 10
NCOMP = 24
NCLS = 21
SCALE = 6.5131 / 40.0
BATCH = 32768
NCORES = 8
CB = BATCH // NCORES            # 4096 rows per core
CELLS = CB * S * S              # 409600 cells per core
P = 128
CPP = CELLS // P                # 3200 cells per partition
K = 200                         # cells per partition per tile
NT = CPP // K                   # tiles
PF = 3                          # DMA prefetch depth
# deferred-Ln chunks: (start_cell, n_cells, emit_at_iter); -1 = epilogue
LN_CHUNKS = [(0, 1600, 9), (1600, 1600, 16)]

_CACHE = {}


def _build_nc():
    import concourse.bacc as bacc
    import concourse.tile as tile
    import concourse.mybir as mybir

    f32 = mybir.dt.float32
    bf16 = mybir.dt.bfloat16
    AF = mybir.ActivationFunctionType
    OP = mybir.AluOpType

    nc = bacc.Bacc("TRN2", target_bir_lowering=False, debug=False)
    pred = nc.dram_tensor("pred", [CB, S, S, NCOMP], f32, kind="ExternalInput").ap()
    tgt = nc.dram_tensor("tgt", [CB, S, S, 4], f32, kind="ExternalInput").ap()
    out = nc.dram_tensor("out", [P, 8], f32, kind="ExternalOutput").ap()

    p_t = pred.flatten_outer_dims().rearrange("(t p k) c -> t p k c", p=P, k=K)
    g_t = tgt.flatten_outer_dims().rearrange("(t p k) c -> t p k c", p=P, k=K)

    with tile.TileContext(nc) as tc:
        with (
            tc.tile_pool(name="singles", bufs=1) as singles,
            tc.tile_pool(name="io", bufs=PF) as io,
            tc.tile_pool(name="gio", bufs=PF + 2) as gio,
            tc.tile_pool(name="duop", bufs=3) as duop,
            tc.tile_pool(name="mid", bufs=2) as mid,
            tc.tile_pool(name="mid1", bufs=1) as mid1,
            tc.tile_pool(name="mid3", bufs=3) as mid3,
            tc.tile_pool(name="mid5", bufs=5) as mid5,
        ):
            # ---- constants / accumulators / staging (persistent) ----
            massf = singles.tile([P, NCLS], f32)
            for c in range(NCLS):
                nc.vector.memset(massf[:, c:c + 1], 2.0 + 0.5 * c)
            massb = singles.tile([P, NCLS], bf16)
            nc.scalar.activation(massb, massf, AF.Copy)
            neg10 = singles.tile([P, 1], f32)
            nc.vector.memset(neg10, -10.0)
            half = singles.tile([P, 1], f32)
            nc.vector.memset(half, 0.5)
            neg1 = singles.tile([P, 1], f32)
            nc.vector.memset(neg1, -1.0)
            acc = singles.tile([P, 8], f32)
            nc.vector.memset(acc, 0.0)
            accE = singles.tile([P, NT], f32)
            accXY = singles.tile([P, NT], f32)
            accOBJ = singles.tile([P, NT], f32)
            accA = singles.tile([P, NT], f32)
            accR = singles.tile([P, NT], f32)
            accWH = singles.tile([P, 4], f32)
            wp_all = singles.tile([P, CPP], bf16)
            wt_all = singles.tile([P, CPP], bf16)
            cf_all = singles.tile([P, CPP], bf16)
            trashB = singles.tile([P, 1600], f32)
            trashA = singles.tile([P, K], f32)

            h = {}  # (name, t) -> tile handle

            def dma(t):
                pt = io.tile([P, K, NCOMP], f32, tag="pt")
                gt = gio.tile([P, K, 4], f32, tag="gt")
                nc.sync.dma_start(out=gt, in_=g_t[t])
                nc.sync.dma_start(out=pt, in_=p_t[t])
                h["pt", t] = pt
                h["gt", t] = gt

            # ---------------- ACT stream pieces ----------------
            def act_s0(t):
                pt, gt = h["pt", t], h["gt", t]
                xc = pt[:, :, 0:4].rearrange("p k (a b) -> p k a b", a=2)[:, :, :, 0]
                tg2 = gt[:, :, 0:4].rearrange("p k (a b) -> p k a b", a=2)[:, :, :, 0]
                duo = duop.tile([P, K, 2 * NCLS], bf16, tag="duo")
                nc.scalar.activation(duo[:, :, 0:NCLS], pt[:, :, 3:24], AF.Exp)
                h["duo", t] = duo

            def act_s0b(t):
                pt = h["pt", t]
                xc = pt[:, :, 0:4].rearrange("p k (a b) -> p k a b", a=2)[:, :, :, 0]
                th2 = mid.tile([P, K, 2], bf16, tag="th2")
                nc.scalar.activation(th2, xc, AF.Tanh, scale=0.5)
                nc.scalar.activation(wp_all[:, t * K:(t + 1) * K],
                                     pt[:, :, 1], AF.Copy)
                h["th2", t] = th2

            def pool_wt(t):
                nc.gpsimd.tensor_scalar_add(wt_all[:, t * K:(t + 1) * K],
                                            h["gt", t][:, :, 1], 0.0)

            def act_e2(t):
                # conf*e = (0.5*conf*(y+1))^2 with y = tanh(v/2) - 2t  (conf
                # binary); x2 = conf*(y+1) from dve_x2
                x2, y2 = h["x2", t], h["q2", t]
                nc.scalar.activation(trashA, x2[:, :, 0], AF.Square, scale=0.5,
                                     accum_out=accXY[:, t:t + 1])
                nc.scalar.activation(trashA, x2[:, :, 1], AF.Square, scale=0.5,
                                     accum_out=accOBJ[:, t:t + 1])
                nc.scalar.activation(trashA, y2[:, :, 1], AF.Square, scale=0.5,
                                     bias=half[:, 0:1],
                                     accum_out=accE[:, t:t + 1])

            def act_absd(t):
                absd = mid3.tile([P, K], f32, tag="absd")
                nc.scalar.activation(absd, h["d0", t], AF.Abs, scale=10.0,
                                     bias=neg10[:, 0:1])
                h["absd", t] = absd

            def act_relu(t):
                r_ = mid.tile([P, K], bf16, tag="r_")
                nc.scalar.activation(r_, h["am", t], AF.Relu, bias=neg1[:, 0:1])
                h["r_", t] = r_

            # ---------------- Pool stream pieces ----------------
            def pool_s0(t):
                gt = h["gt", t]
                nc.gpsimd.tensor_scalar_add(cf_all[:, t * K:(t + 1) * K],
                                            gt[:, :, 2], 0.0)
                g_ = mid.tile([P, K], f32, tag="g_")
                nc.vector.tensor_scalar_min(g_, gt[:, :, 3], 1.0)
                cm = mid5.tile([P, K], f32, tag="cm")
                nc.gpsimd.tensor_mul(cm, g_, gt[:, :, 2])
                h["cm", t] = cm
                ctp1 = mid.tile([P, K], f32, tag="ctp1")
                nc.gpsimd.tensor_scalar_add(ctp1, gt[:, :, 3], 1.0)
                h["ctp1", t] = ctp1

            def dve_q2(t):
                # y = tanh(v/2) - 2t
                gt = h["gt", t]
                tg2 = gt[:, :, 0:4].rearrange("p k (a b) -> p k a b", a=2)[:, :, :, 0]
                q2 = mid.tile([P, K, 2], bf16, tag="q2")
                nc.vector.scalar_tensor_tensor(
                    q2, tg2, -2.0, h["th2", t], op0=OP.mult, op1=OP.add
                )
                h["q2", t] = q2
                confb = gt[:, :, 2].unsqueeze(-1).broadcast_to([P, K, 2])
                x2 = mid.tile([P, K, 2], bf16, tag="x2")
                nc.vector.scalar_tensor_tensor(
                    x2, q2, 1.0, confb, op0=OP.add, op1=OP.mult
                )
                h["x2", t] = x2

            def pool_tree(t):
                v, B, gt = h["v", t], h["B", t], h["gt", t]
                C = mid.tile([P, K, 2, 2], bf16, tag="C")
                nc.gpsimd.tensor_add(C, B[:, :, :, 0:2], B[:, :, :, 2:4])
                D = mid.tile([P, K, 2], bf16, tag="D")
                nc.gpsimd.tensor_add(D, C[:, :, :, 0], C[:, :, :, 1])
                E = mid.tile([P, K, 2], bf16, tag="E")
                nc.gpsimd.tensor_add(E, D, B[:, :, :, 4])
                ST = mid.tile([P, K, 2], bf16, tag="ST")
                nc.gpsimd.tensor_add(ST, E, v[:, :, :, 20])
                sp = mid.tile([P, K], f32, tag="sp")
                nc.gpsimd.tensor_mul(sp, h["ctp1", t], ST[:, :, 0])
                h["ST", t] = ST
                h["sp", t] = sp

            def pool_d0(t):
                d0 = mid.tile([P, K], f32, tag="d0")
                nc.gpsimd.tensor_mul(d0, h["ST", t][:, :, 1], h["rsp", t])
                h["d0", t] = d0

            def pool_am(t):
                am = mid.tile([P, K], bf16, tag="am")
                nc.gpsimd.tensor_mul(am, h["absd", t], h["cm", t])
                h["am", t] = am

            # ---------------- DVE stream pieces ----------------
            def dve_s0(t):
                duo = h["duo", t]
                v = duo.rearrange("p k (a c) -> p k a c", a=2)
                nc.vector.tensor_mul(
                    duo[:, :, NCLS:2 * NCLS], duo[:, :, 0:NCLS],
                    massb.unsqueeze(1).broadcast_to([P, K, NCLS]),
                )
                A = mid1.tile([P, K, 2, 10], bf16, tag="A")
                nc.vector.tensor_add(A, v[:, :, :, 0:10], v[:, :, :, 10:20])
                B = mid.tile([P, K, 2, 5], bf16, tag="B")
                nc.vector.tensor_add(B, A[:, :, :, 0:5], A[:, :, :, 5:10])
                h["v", t] = v
                h["B", t] = B

            def dve_recip(t):
                rsp = mid.tile([P, K], f32, tag="rsp")
                nc.vector.reciprocal_approx_fast(rsp, h["sp", t])
                h["rsp", t] = rsp

            def dve_huber(t):
                # 0.5*sum(cm*a^2) - 0.5*sum(r^2): cm binary so cm*a^2 = am^2
                am, r_ = h["am", t], h["r_", t]
                amsq = mid.tile([P, K], bf16, tag="amsq")
                nc.vector.tensor_mul(amsq, am, am)
                nc.vector.reduce_sum(accA[:, t:t + 1], amsq,
                                     axis=mybir.AxisListType.X)
                rsq = mid.tile([P, K], bf16, tag="rsq")
                nc.vector.tensor_mul(rsq, r_, r_)
                nc.vector.reduce_sum(accR[:, t:t + 1], rsq,
                                     axis=mybir.AxisListType.X)

            def ln_phase_a(lo, n, bidx):
                lnw_t = mid1.tile([P, 1600], bf16, tag="lnw")
                lnw = lnw_t[:, 0:n]
                nc.scalar.activation(lnw, wt_all[:, lo:lo + n], AF.Ln,
                                     scale=1.0 / SCALE)
                dwm_t = mid1.tile([P, 1600], bf16, tag="dwm")
                dwm = dwm_t[:, 0:n]
                nc.vector.tensor_sub(dwm, wp_all[:, lo:lo + n], lnw)
                vw_t = mid1.tile([P, 1600], bf16, tag="vw")
                vw = vw_t[:, 0:n]
                nc.vector.tensor_mul(vw, dwm, cf_all[:, lo:lo + n])
                h["vw", bidx] = vw

            def ln_phase_b(n, bidx):
                # conf binary: sum((dw*conf)^2) = sum(conf*dw^2); Square is in
                # every ACT table set so this forces no table load
                nc.scalar.activation(trashB[:, 0:n], h["vw", bidx], AF.Square,
                                     accum_out=accWH[:, bidx:bidx + 1])

            # ---------------- software-pipelined main loop ----------------
            # stage offsets (tile t work emitted at iter t+off):
            #   0: dma-prefetch, pool cfb/cm, act exp/tanh/u2/wp/wt, dve prod/L1/L2
            #   1: pool q2/tree/sp, act e2/sqc (after exp)
            #   2: dve xyobj/recip, pool d0
            #   3: act absd
            #   4: pool am, act relu
            #   5: dve huber
            # first pred tile first: exp(0) is the critical path at startup
            pt0 = io.tile([P, K, NCOMP], f32, tag="pt")
            nc.sync.dma_start(out=pt0, in_=p_t[0])
            h["pt", 0] = pt0
            gt0 = gio.tile([P, K, 4], f32, tag="gt")
            nc.sync.dma_start(out=gt0, in_=g_t[0])
            h["gt", 0] = gt0
            for t in range(1, PF):
                dma(t)
            for u in range(NT + 5):
                if u + PF < NT:
                    dma(u + PF)
                if u < NT:
                    act_s0(u)                      # exp first: longest ACT op
                if u >= 1 and u - 1 < NT:
                    dve_q2(u - 1)
                    act_e2(u - 1)
                if u >= 4 and u - 4 < NT:
                    dve_huber(u - 4)
                if u >= 2 and u - 2 < NT:
                    dve_recip(u - 2)
                if u < NT:
                    pool_s0(u)
                    pool_wt(u)
                if u >= 3 and u - 3 < NT:
                    pool_am(u - 3)
                if u >= 1 and u - 1 < NT:
                    pool_tree(u - 1)
                if u >= 2 and u - 2 < NT:
                    pool_d0(u - 2)
                if u < NT:
                    act_s0b(u)
                if u >= 2 and u - 2 < NT:
                    act_absd(u - 2)
                if u >= 3 and u - 3 < NT:
                    act_relu(u - 3)
                if u < NT:
                    dve_s0(u)
                for bidx, (lo, n, at) in enumerate(LN_CHUNKS):
                    if at == u:
                        ln_phase_a(lo, n, bidx)
                    if at == u - 1:
                        ln_phase_b(n, bidx)

            for bidx, (lo, n, at) in enumerate(LN_CHUNKS):
                if at < 0:
                    ln_phase_a(lo, n, bidx)
                    ln_phase_b(n, bidx)
            # fold per-tile accumulator columns into acc slots
            AX = mybir.AxisListType.X
            nc.vector.reduce_sum(acc[:, 0:1], accXY, axis=AX)
            nc.vector.reduce_sum(acc[:, 1:2], accWH[:, 0:len(LN_CHUNKS)],
                                 axis=AX)
            nc.vector.reduce_sum(acc[:, 2:3], accOBJ, axis=AX)
            nc.vector.reduce_sum(acc[:, 3:4], accE, axis=AX)
            nc.vector.reduce_sum(acc[:, 4:5], accA, axis=AX)
            nc.vector.reduce_sum(acc[:, 5:6], accR, axis=AX)

            nc.sync.dma_start(out=out, in_=acc)
    nc.finalize()
    return nc


def _get_nc():
    if "nc" not in _CACHE:
        _CACHE["nc"] = _build_nc()
    return _CACHE["nc"]


def run_sharded(pred_tensor, target_tensor, trace=False):
    from concourse.bass_utils import run_bass_kernel_spmd

    nc = _get_nc()
    pred_np = np.ascontiguousarray(np.asarray(pred_tensor, dtype=np.float32))
    tgt_np = np.ascontiguousarray(np.asarray(target_tensor, dtype=np.float32))
    in_maps = [
        {
            "pred": pred_np[i * CB : (i + 1) * CB],
            "tgt": tgt_np[i * CB : (i + 1) * CB],
        }
        for i in range(NCORES)
    ]
    res = run_bass_kernel_spmd(nc, in_maps, core_ids=list(range(NCORES)), trace=trace)
    return [r["out"] for r in res.results], res


def kernel(pred_tensor, target_tensor):
    partials, _ = run_sharded(pred_tensor, target_tensor, trace=False)
    tot = np.zeros(8, dtype=np.float64)
    for p in partials:
        tot += p.astype(np.float64).sum(axis=0)
    xy, wh, obj, e_tot = tot[0], tot[1], tot[2], tot[3]
    cls = 0.5 * (tot[4] - tot[5])
    noobj = e_tot - obj
    loss = 10.0 * (xy + wh) + obj + 1.0 * noobj + 0.5 * cls
    inv = 1.0 / BATCH
    return np.array(
        [xy * inv, wh * inv, obj * inv, noobj * inv, cls * inv, loss * inv],
        dtype=np.float32,
    )

